# revision 1
# baseline (speedup 1.0000x reference)
import sys

sys.path.insert(0, "/opt/trn_rl_repo")
import numpy as np
from concourse import bass, bacc, tile, mybir
from concourse.bass_utils import run_bass_kernel_spmd
from concourse.masks import make_identity
from concourse.tile import add_dep_helper

fp32 = mybir.dt.float32
bf16 = mybir.dt.bfloat16
u32 = mybir.dt.uint32
u16 = mybir.dt.uint16
fp16 = mybir.dt.float16
u8 = mybir.dt.uint8

SEQ = 12
HALF = 6
N = 512
K = 8
NCHUNK = 4  # 512 queries / 128
BPC = 2  # batches per core
NCORES = 8
CTOT = 448  # 64+128+256
R1SQ = float(np.float32(4.0 + 1e-6) * np.float32(4.0 + 1e-6))
H = 64  # motion MLP hidden
BIG = 3.0e4  # -BIG marks out-of-radius neighbors before the max

_CACHE = {}


def _build():
    if "nc" in _CACHE:
        return _CACHE["nc"]
    nc = bacc.Bacc(target_bir_lowering=False)

    # host-precomputed per-frame transposed tensors
    q5_l = nc.dram_tensor("q5_l", (BPC * SEQ, 5, N), fp32, kind="ExternalInput")
    k5_l = nc.dram_tensor("k5_l", (BPC * SEQ, 5, N), fp32, kind="ExternalInput")
    q4_l = nc.dram_tensor("q4_l", (BPC * SEQ, 4, N), bf16, kind="ExternalInput")
    # weights (bf16 feature path)
    WB1 = nc.dram_tensor("WB1", (4, 64), bf16, kind="ExternalInput")
    WB2 = nc.dram_tensor("WB2", (4, 128), bf16, kind="ExternalInput")
    WB3 = nc.dram_tensor("WB3", (4, 256), bf16, kind="ExternalInput")
    Wnf1 = nc.dram_tensor("Wnf1", (64, 64), bf16, kind="ExternalInput")
    Wnf2 = nc.dram_tensor("Wnf2", (128, 128), bf16, kind="ExternalInput")
    Wnf3 = nc.dram_tensor("Wnf3", (256, 256), bf16, kind="ExternalInput")
    CW1 = nc.dram_tensor("CW1", (3, 64), bf16, kind="ExternalInput")
    CW2 = nc.dram_tensor("CW2", (3, 128), bf16, kind="ExternalInput")
    CW3 = nc.dram_tensor("CW3", (3, 256), bf16, kind="ExternalInput")
    Wfi2 = nc.dram_tensor("Wfi2", (64, 128), bf16, kind="ExternalInput")
    Wfi3 = nc.dram_tensor("Wfi3", (128, 256), bf16, kind="ExternalInput")
    Wm = nc.dram_tensor("Wm", (256, H), bf16, kind="ExternalInput")
    Wl = nc.dram_tensor("Wl", (H, 3), bf16, kind="ExternalInput")
    bmT = nc.dram_tensor("bmT", (H, 1), fp32, kind="ExternalInput")
    blT = nc.dram_tensor("blT", (3, 1), fp32, kind="ExternalInput")

    preds = nc.dram_tensor("preds", (BPC * HALF * 3, N), fp32, kind="ExternalOutput")
    tabs = [
        [nc.dram_tensor(f"tab_b{b}_p{p}", (N, CTOT), bf16) for p in range(2)]
        for b in range(BPC)
    ]

    with tile.TileContext(nc) as tc:
        with tc.tile_pool(name="sb", bufs=1) as sb, tc.tile_pool(
            name="ps", bufs=1, space="PSUM"
        ) as ps:
            # ---- persistent weights in SBUF ----
            wb1_t = sb.tile([4, 64], bf16, tag="wb1")
            wb2_t = sb.tile([4, 128], bf16, tag="wb2")
            wb3_t = sb.tile([4, 256], bf16, tag="wb3")
            wnf1_t = sb.tile([64, 64], bf16, tag="wnf1")
            wnf2_t = sb.tile([128, 128], bf16, tag="wnf2")
            wnf3a_t = sb.tile([128, 256], bf16, tag="wnf3a")
            wnf3b_t = sb.tile([128, 256], bf16, tag="wnf3b")
            cw1_t = sb.tile([3, 64], bf16, tag="cw1")
            cw2_t = sb.tile([3, 128], bf16, tag="cw2")
            cw3_t = sb.tile([3, 256], bf16, tag="cw3")
            wfi2_t = sb.tile([64, 128], bf16, tag="wfi2")
            wfi3_t = sb.tile([128, 256], bf16, tag="wfi3")
            wma_t = sb.tile([128, H], bf16, tag="wma")
            wmb_t = sb.tile([128, H], bf16, tag="wmb")
            wl_t = sb.tile([H, 3], bf16, tag="wl")
            bmT_t = sb.tile([H, 1], fp32, tag="bmT")
            blT_t = sb.tile([3, 1], fp32, tag="blT")
            ident = sb.tile([128, 128], fp32, tag="ident")
            onec3 = sb.tile([3, 1], fp32, tag="onec3")
            nc.sync.dma_start(wb1_t[:], WB1[:])
            nc.sync.dma_start(wb2_t[:], WB2[:])
            nc.sync.dma_start(wb3_t[:], WB3[:])
            nc.sync.dma_start(wnf1_t[:], Wnf1[:])
            nc.sync.dma_start(wnf2_t[:], Wnf2[:])
            nc.sync.dma_start(wnf3a_t[:], Wnf3[0:128, :])
            nc.sync.dma_start(wnf3b_t[:], Wnf3[128:256, :])
            nc.sync.dma_start(cw1_t[:], CW1[:])
            nc.sync.dma_start(cw2_t[:], CW2[:])
            nc.sync.dma_start(cw3_t[:], CW3[:])
            nc.sync.dma_start(wfi2_t[:], Wfi2[:])
            nc.sync.dma_start(wfi3_t[:], Wfi3[:])
            nc.sync.dma_start(wma_t[:], Wm[0:128, :])
            nc.sync.dma_start(wmb_t[:], Wm[128:256, :])
            nc.sync.dma_start(wl_t[:], Wl[:])
            nc.sync.dma_start(bmT_t[:], bmT[:])
            nc.sync.dma_start(blT_t[:], blT[:])
            make_identity(nc, ident[:])
            nc.vector.memset(onec3[:], -1.0)  # for -|q|^2 row sums

            # per-batch state tiles; t loop outer so the two batch chains interleave
            q5_b, q4_b, key5_b, fALL_b, aux_b = [], [], [], [], []
            for b in range(BPC):
                q5_b.append([sb.tile([5, N], fp32, tag=f"q5_{b}_{i}", name=f"q5_{b}_{i}") for i in range(2)])
                q4_b.append([sb.tile([4, N], bf16, tag=f"q4_{b}_{i}", name=f"q4_{b}_{i}") for i in range(2)])
                key5_b.append(sb.tile([5, N], fp32, tag=f"key5_{b}", name=f"key5_{b}"))
                fALL_b.append(sb.tile([128, 4, N], bf16, tag=f"fALL_{b}", name=f"fALL_{b}"))
                nc.vector.memset(fALL_b[b][:], 0.0)

            def qidx(t):
                return t % 2 if t < HALF else (t + 1) % 2

            for t in range(SEQ):
                for b in range(BPC):
                    q5, q4 = q5_b[b], q4_b[b]
                    key5, fALL = key5_b[b], fALL_b[b]
                    tab = tabs[b][t % 2]
                    qi = qidx(t)
                    qt5, qt4 = q5[qi], q4[qi]
                    if t < HALF:
                        base = b * SEQ + t
                        nc.sync.dma_start(qt5[:], q5_l[base, :, :])
                        nc.sync.dma_start(qt4[:], q4_l[base, :, :])
                        kbase = b * SEQ + max(t - 1, 0)
                        nc.sync.dma_start(key5[:], k5_l[kbase, :, :])
                        kt4 = q4[max(t - 1, 0) % 2]
                    elif t == HALF:
                        # q stays = frame5 tiles; keys = frame5 too
                        nc.sync.dma_start(key5[:], k5_l[b * SEQ + HALF - 1, :, :])
                        kt4 = q4[(HALF - 1) % 2]
                    else:
                        # key tiles derived at the end of step t-1
                        kt4 = q4[qidx(t - 1)]

                    # ---- A table: A[key] = [k;1]@WB + f@Wnf, per key chunk ----
                    wr_insts = []
                    for j in range(NCHUNK):
                        jj = slice(j * 128, (j + 1) * 128)
                        a_ps = ps.tile([128, CTOT], fp32, tag="a_ps", bufs=2)
                        nc.tensor.matmul(
                            a_ps[:, 0:64], kt4[:, jj], wb1_t[:], start=True, stop=False
                        )
                        nc.tensor.matmul(
                            a_ps[:, 0:64], fALL[0:64, 0, jj], wnf1_t[:],
                            start=False, stop=True,
                        )
                        nc.tensor.matmul(
                            a_ps[:, 64:192], kt4[:, jj], wb2_t[:], start=True, stop=False
                        )
                        nc.tensor.matmul(
                            a_ps[:, 64:192], fALL[:, 1, jj], wnf2_t[:],
                            start=False, stop=True,
                        )
                        nc.tensor.matmul(
                            a_ps[:, 192:448], kt4[:, jj], wb3_t[:], start=True, stop=False
                        )
                        nc.tensor.matmul(
                            a_ps[:, 192:448], fALL[:, 2, jj], wnf3a_t[:],
                            start=False, stop=False,
                        )
                        nc.tensor.matmul(
                            a_ps[:, 192:448], fALL[:, 3, jj], wnf3b_t[:],
                            start=False, stop=True,
                        )
                        a_sb = sb.tile([128, CTOT], bf16, tag="a_sb", bufs=4)
                        nc.scalar.copy(a_sb[:], a_ps[:])
                        w = nc.sync.dma_start(tab[jj, :], a_sb[:])
                        wr_insts.append(w.ins)

                    # ---- per query chunk ----
                    for j in range(NCHUNK):
                        jj = slice(j * 128, (j + 1) * 128)
                        # -d2 = 2q.k - |q|^2 - |k|^2 via one packed matmul
                        d2_ps = ps.tile([128, N], fp32, tag="d2_ps", bufs=3)
                        nc.tensor.matmul(
                            d2_ps[:], qt5[:, jj], key5[:], start=True, stop=True
                        )
                        d2h = sb.tile([128, N], fp16, tag="d2h", bufs=4)
                        nc.scalar.copy(d2h[:], d2_ps[:])
                        vals = sb.tile([128, K], fp16, tag="vals", bufs=4)
                        idx = sb.tile([128, K], u32, tag="idx", bufs=4)
                        nc.vector.max(vals[:], d2h[:])
                        nc.vector.max_index(idx[:], vals[:], d2h[:])
                        # addend = (vals < -r^2) * -BIG  (0 for valid)
                        addend = sb.tile([128, K, 1], bf16, tag="addend", bufs=4)
                        nc.vector.tensor_scalar(
                            addend[:, :, 0], vals[:], -R1SQ, -BIG,
                            op0=mybir.AluOpType.is_lt, op1=mybir.AluOpType.mult,
                        )
                        # fused gather of all K neighbor rows
                        g = sb.tile([128, K, CTOT], bf16, tag="g", bufs=4)
                        for k in range(K):
                            gi = nc.gpsimd.indirect_dma_start(
                                out=g[:, k, :],
                                out_offset=None,
                                in_=tab[:],
                                in_offset=bass.IndirectOffsetOnAxis(
                                    ap=idx[:, k : k + 1], axis=0
                                ),
                            )
                            for w in wr_insts:
                                add_dep_helper(gi.ins, w, reason="gather after tab write")

                        m_sb = sb.tile([128, CTOT], fp32, tag="m_sb", bufs=4)
                        # cell1: mask out-of-radius entries, max, then include k=0
                        g1m = sb.tile([128, K, 64], bf16, tag="g1m", bufs=4)
                        nc.vector.tensor_tensor(
                            g1m[:],
                            g[:, :, 0:64],
                            addend[:].broadcast_to((128, K, 64)),
                            op=mybir.AluOpType.add,
                        )
                        p1a = sb.tile([128, 4, 64], bf16, tag="p1a", bufs=4)
                        nc.vector.tensor_tensor(
                            p1a[:], g1m[:, 0:4, :], g1m[:, 4:8, :],
                            op=mybir.AluOpType.max,
                        )
                        p1b = sb.tile([128, 2, 64], bf16, tag="p1b", bufs=4)
                        nc.vector.tensor_tensor(
                            p1b[:], p1a[:, 0:2, :], p1a[:, 2:4, :],
                            op=mybir.AluOpType.max,
                        )
                        p1c = sb.tile([128, 64], bf16, tag="p1c", bufs=4)
                        nc.vector.tensor_tensor(
                            p1c[:], p1b[:, 0, :], p1b[:, 1, :],
                            op=mybir.AluOpType.max,
                        )
                        nc.vector.tensor_tensor(
                            m_sb[:, 0:64], p1c[:], g[:, 0, 0:64],
                            op=mybir.AluOpType.max,
                        )
                        # cells 2+3: plain max pyramid over K
                        p2a = sb.tile([128, 4, 384], bf16, tag="p2a", bufs=4)
                        nc.vector.tensor_tensor(
                            p2a[:], g[:, 0:4, 64:448], g[:, 4:8, 64:448],
                            op=mybir.AluOpType.max,
                        )
                        p2b = sb.tile([128, 2, 384], bf16, tag="p2b", bufs=4)
                        nc.vector.tensor_tensor(
                            p2b[:], p2a[:, 0:2, :], p2a[:, 2:4, :],
                            op=mybir.AluOpType.max,
                        )
                        nc.vector.tensor_tensor(
                            m_sb[:, 64:448], p2b[:, 0, :], p2b[:, 1, :],
                            op=mybir.AluOpType.max,
                        )

                        # ---- transposed C + m^T accumulate, per cell ----
                        ct = ps.tile([128, 4, 128], fp32, tag="ct", bufs=2)
                        # cell1 (cols 0:64 live; 64:128 zeroed via CW1z padding)
                        nc.tensor.matmul(
                            ct[0:64, 0, :], cw1_t[:], qt4[0:3, jj], start=True, stop=False
                        )
                        nc.tensor.matmul(
                            ct[0:64, 0, :], m_sb[:, 0:64], ident[:],
                            is_transpose=True, start=False, stop=True,
                        )
                        f1c = nc.scalar.copy(fALL[0:64, 0, jj], ct[0:64, 0, :])
                        # cell2 (cols 64:192)
                        nc.tensor.matmul(
                            ct[:, 1, :], cw2_t[:], qt4[0:3, jj], start=True, stop=False
                        )
                        nc.tensor.matmul(
                            ct[:, 1, :], wfi2_t[:], fALL[0:64, 0, jj],
                            start=False, stop=False,
                        )
                        nc.tensor.matmul(
                            ct[:, 1, :], m_sb[:, 64:192], ident[:],
                            is_transpose=True, start=False, stop=True,
                        )
                        nc.scalar.copy(fALL[:, 1, jj], ct[:, 1, :])
                        # cell3 (cols 192:448)
                        nc.tensor.matmul(
                            ct[:, 2, :], cw3_t[:, 0:128], qt4[0:3, jj],
                            start=True, stop=False,
                        )
                        nc.tensor.matmul(
                            ct[:, 2, :], wfi3_t[:, 0:128], fALL[:, 1, jj],
                            start=False, stop=False,
                        )
                        nc.tensor.matmul(
                            ct[:, 2, :], m_sb[:, 192:320], ident[:],
                            is_transpose=True, start=False, stop=True,
                        )
                        nc.tensor.matmul(
                            ct[:, 3, :], cw3_t[:, 128:256], qt4[0:3, jj],
                            start=True, stop=False,
                        )
                        nc.tensor.matmul(
                            ct[:, 3, :], wfi3_t[:, 128:256], fALL[:, 1, jj],
                            start=False, stop=False,
                        )
                        nc.tensor.matmul(
                            ct[:, 3, :], m_sb[:, 320:448], ident[:],
                            is_transpose=True, start=False, stop=True,
                        )
                        nc.scalar.copy(fALL[:, 2:4, jj], ct[:, 2:4, :])

                    if t >= HALF:
                        # motion MLP from f3 (fALL slices 2:4)
                        aux = ps.tile([128, N], fp32, tag="aux", bufs=1)
                        nc.tensor.matmul(
                            aux[0:H, :], wma_t[:], fALL[:, 2, :], start=True, stop=False
                        )
                        nc.tensor.matmul(
                            aux[0:H, :], wmb_t[:], fALL[:, 3, :], start=False, stop=True
                        )
                        hm_sb = sb.tile([H, N], bf16, tag=f"hm_sb_{b}")
                        nc.scalar.activation(
                            hm_sb[:], aux[0:H, :],
                            mybir.ActivationFunctionType.Relu,
                            bias=bmT_t[:], scale=1.0,
                        )
                        nc.tensor.matmul(
                            aux[0:3, :], wl_t[:], hm_sb[:], start=True, stop=True
                        )
                        # next query = q + motion + bl
                        nq5 = q5[qidx(t + 1) if t + 1 < SEQ else (t % 2)]
                        nq4 = q4[qidx(t + 1) if t + 1 < SEQ else (t % 2)]
                        nc.vector.scalar_tensor_tensor(
                            nq5[0:3, :], aux[0:3, :], blT_t[:], qt5[0:3, :],
                            op0=mybir.AluOpType.add, op1=mybir.AluOpType.add,
                        )
                        row = (b * HALF + (t - HALF)) * 3
                        nc.sync.dma_start(preds[row : row + 3, :], nq5[0:3, :])
                        if t + 1 < SEQ:
                            # derive next-step q aux rows and key tiles
                            sq3 = sb.tile([3, N], fp32, tag=f"sq3_{b}", bufs=2)
                            nc.vector.tensor_tensor(
                                sq3[:], nq5[0:3, :], nq5[0:3, :],
                                op=mybir.AluOpType.mult,
                            )
                            nc.tensor.matmul(
                                aux[0:1, :], onec3[:], sq3[:], start=True, stop=True
                            )
                            nqs_sb = sb.tile([1, N], fp32, tag="nqs_sb", bufs=2)
                            nc.scalar.copy(nqs_sb[:], aux[0:1, :])
                            nc.sync.dma_start(nq5[4:5, :], nqs_sb[:])
                            nc.scalar.copy(nq4[0:3, :], nq5[0:3, :])
                            # key tiles for t+1 = current q (scaled forms)
                            nc.vector.tensor_scalar(
                                key5[0:3, :], qt5[0:3, :], 2.0, None,
                                op0=mybir.AluOpType.mult,
                            )
                            nc.sync.dma_start(key5[3:4, :], qt5[4:5, :])

    nc.finalize()
    _CACHE["nc"] = nc
    return nc


def _prep_weights(inputs):
    W1, b1 = inputs["W1"], inputs["b1"]
    W2, b2 = inputs["W2"], inputs["b2"]
    W3, b3 = inputs["W3"], inputs["b3"]

    def wb(W, bvec, cout):
        return np.ascontiguousarray(
            np.concatenate([W[0:3], bvec[None, :]], axis=0), np.float32
        )


    return {
        "WB1": wb(W1, b1, 64),
        "WB2": wb(W2, b2, 128),
        "WB3": wb(W3, b3, 256),
        "Wnf1": np.ascontiguousarray(W1[3:67], np.float32),
        "Wnf2": np.ascontiguousarray(W2[67:195], np.float32),
        "Wnf3": np.ascontiguousarray(W3[131:387], np.float32),
        "CW1": np.ascontiguousarray(-W1[0:3], np.float32),
        "CW2": np.ascontiguousarray(-W2[0:3], np.float32),
        "CW3": np.ascontiguousarray(-W3[0:3], np.float32),
        "Wfi2": np.ascontiguousarray(W2[3:67], np.float32),
        "Wfi3": np.ascontiguousarray(W3[3:131], np.float32),
        "Wm": np.ascontiguousarray(inputs["Wm"], np.float32),
        "Wl": np.ascontiguousarray(inputs["Wl"], np.float32),
        "bmT": np.ascontiguousarray(inputs["bm"][:, None], np.float32),
        "blT": np.ascontiguousarray(inputs["bl"][:, None], np.float32),
    }


def _to_bf16(x):
    import ml_dtypes

    return np.asarray(x, np.float32).astype(ml_dtypes.bfloat16)


def _prep_frames(frames):
    # frames (BPC, SEQ, N, 3) for one core -> q5/k5 fp32 and q4 bf16 rows
    x = frames.transpose(0, 1, 3, 2)  # (BPC, SEQ, 3, N)
    ssq = np.sum(x * x, axis=2, keepdims=True)  # (BPC, SEQ, 1, N)
    ones = np.ones_like(ssq)
    q5 = np.concatenate([x, ones, -ssq], axis=2).reshape(BPC * SEQ, 5, N)
    k5 = np.concatenate([2.0 * x, -ssq, ones], axis=2).reshape(BPC * SEQ, 5, N)
    q4 = np.concatenate([x, ones], axis=2).reshape(BPC * SEQ, 4, N)
    return (
        np.ascontiguousarray(q5, np.float32),
        np.ascontiguousarray(k5, np.float32),
        _to_bf16(np.ascontiguousarray(q4, np.float32)),
    )


def _run(inputs, **spmd_kwargs):
    nc = _build()
    frames = np.asarray(inputs["frames"], np.float32)  # (16,12,512,3)
    wd = _prep_weights(
        {k: np.asarray(v, np.float32) for k, v in inputs.items() if k != "frames"}
    )
    bf_names = {
        "WB1", "WB2", "WB3", "Wnf1", "Wnf2", "Wnf3", "CW1", "CW2", "CW3",
        "Wfi2", "Wfi3", "Wm", "Wl",
    }
    wd = {k: (_to_bf16(v) if k in bf_names else v) for k, v in wd.items()}
    in_maps = []
    for c in range(NCORES):
        d = dict(wd)
        q5c, k5c, q4c = _prep_frames(frames[c * BPC : (c + 1) * BPC])
        d["q5_l"] = q5c
        d["k5_l"] = k5c
        d["q4_l"] = q4c
        in_maps.append(d)
    res = run_bass_kernel_spmd(nc, in_maps, list(range(NCORES)), **spmd_kwargs)
    outs = []
    for c in range(NCORES):
        p = np.asarray(res.results[c]["preds"]).reshape(BPC, HALF, 3, N)
        outs.append(p.transpose(0, 1, 3, 2))  # (2,6,512,3)
    return np.ascontiguousarray(np.concatenate(outs, axis=0), np.float32), res


def kernel(**inputs):
    return _run(inputs)[0]



# revision 26
# speedup vs baseline: 1.2459x; 1.2459x over previous
import sys

sys.path.insert(0, "/opt/trn_rl_repo")
import numpy as np
from concourse import bass, bacc, tile, mybir
from concourse.bass_utils import run_bass_kernel_spmd
from concourse.masks import make_identity
from concourse import library_config
from concourse.tile import add_dep_helper

fp32 = mybir.dt.float32
bf16 = mybir.dt.bfloat16
u32 = mybir.dt.uint32
u16 = mybir.dt.uint16
fp16 = mybir.dt.float16
u8 = mybir.dt.uint8

SEQ = 12
HALF = 6
N = 512
K = 8
NCHUNK = 4  # 512 queries / 128
BPC = 2  # batches per core
NCORES = 8
CTOT = 448  # 64+128+256
R1SQ = float(np.float32(4.0 + 1e-6) * np.float32(4.0 + 1e-6))
H = 64  # motion MLP hidden
BIG = 3.0e4  # -BIG marks out-of-radius neighbors before the max

_CACHE = {}


def _build():
    if "nc" in _CACHE:
        return _CACHE["nc"]
    nc = bacc.Bacc(target_bir_lowering=False)

    # host-precomputed per-frame transposed tensors
    q5_l = nc.dram_tensor("q5_l", (BPC * SEQ, 5, N), fp32, kind="ExternalInput")
    k5_l = nc.dram_tensor("k5_l", (BPC * SEQ, 5, N), fp32, kind="ExternalInput")
    q4_l = nc.dram_tensor("q4_l", (BPC * SEQ, 4, N), bf16, kind="ExternalInput")
    # weights (bf16 feature path)
    WB1 = nc.dram_tensor("WB1", (4, 64), bf16, kind="ExternalInput")
    WB2 = nc.dram_tensor("WB2", (4, 128), bf16, kind="ExternalInput")
    WB3 = nc.dram_tensor("WB3", (4, 256), bf16, kind="ExternalInput")
    Wnf1 = nc.dram_tensor("Wnf1", (64, 64), bf16, kind="ExternalInput")
    Wnf2 = nc.dram_tensor("Wnf2", (128, 128), bf16, kind="ExternalInput")
    Wnf3 = nc.dram_tensor("Wnf3", (256, 256), bf16, kind="ExternalInput")
    CW1 = nc.dram_tensor("CW1", (3, 64), bf16, kind="ExternalInput")
    CW2 = nc.dram_tensor("CW2", (3, 128), bf16, kind="ExternalInput")
    CW3 = nc.dram_tensor("CW3", (3, 256), bf16, kind="ExternalInput")
    Wfi2 = nc.dram_tensor("Wfi2", (64, 128), bf16, kind="ExternalInput")
    Wfi3 = nc.dram_tensor("Wfi3", (128, 256), bf16, kind="ExternalInput")
    Wm = nc.dram_tensor("Wm", (256, H), bf16, kind="ExternalInput")
    Wl = nc.dram_tensor("Wl", (H, 3), bf16, kind="ExternalInput")
    bmT = nc.dram_tensor("bmT", (H, 1), fp32, kind="ExternalInput")
    blT = nc.dram_tensor("blT", (3, 1), fp32, kind="ExternalInput")

    preds = nc.dram_tensor("preds", (BPC * HALF * 3, N), fp32, kind="ExternalOutput")
    tabs = [
        [nc.dram_tensor(f"tab_b{b}_p{p}", (N, 512), bf16) for p in range(2)]
        for b in range(BPC)
    ]
    Dts = [
        [nc.dram_tensor(f"D_b{b}_p{p}", (128, 32), u16) for p in range(2)]
        for b in range(BPC)
    ]
    Wts = [
        [nc.dram_tensor(f"W_b{b}_p{p}", (16, 256), u16) for p in range(2)]
        for b in range(BPC)
    ]

    with tile.TileContext(nc) as tc:
        with tc.tile_pool(name="sb", bufs=1) as sb, tc.tile_pool(
            name="ps", bufs=1, space="PSUM"
        ) as ps:
            # ---- persistent weights in SBUF ----
            wb1_t = sb.tile([4, 64], bf16, tag="wb1")
            wb2_t = sb.tile([4, 128], bf16, tag="wb2")
            wb3_t = sb.tile([4, 256], bf16, tag="wb3")
            wnf1_t = sb.tile([64, 64], bf16, tag="wnf1")
            wnf2_t = sb.tile([128, 128], bf16, tag="wnf2")
            wnf3a_t = sb.tile([128, 256], bf16, tag="wnf3a")
            wnf3b_t = sb.tile([128, 256], bf16, tag="wnf3b")
            cw1_t = sb.tile([3, 64], bf16, tag="cw1")
            cw2_t = sb.tile([3, 128], bf16, tag="cw2")
            cw3_t = sb.tile([3, 256], bf16, tag="cw3")
            wfi2_t = sb.tile([64, 128], bf16, tag="wfi2")
            wfi3_t = sb.tile([128, 256], bf16, tag="wfi3")
            wma_t = sb.tile([128, H], bf16, tag="wma")
            wmb_t = sb.tile([128, H], bf16, tag="wmb")
            wl_t = sb.tile([H, 3], bf16, tag="wl")
            bmT_t = sb.tile([H, 1], fp32, tag="bmT")
            blT_t = sb.tile([3, 1], fp32, tag="blT")
            ident = sb.tile([128, 128], fp32, tag="ident")
            onec3 = sb.tile([3, 1], fp32, tag="onec3")
            nc.sync.dma_start(wb1_t[:], WB1[:])
            nc.sync.dma_start(wb2_t[:], WB2[:])
            nc.sync.dma_start(wb3_t[:], WB3[:])
            nc.sync.dma_start(wnf1_t[:], Wnf1[:])
            nc.sync.dma_start(wnf2_t[:], Wnf2[:])
            nc.sync.dma_start(wnf3a_t[:], Wnf3[0:128, :])
            nc.sync.dma_start(wnf3b_t[:], Wnf3[128:256, :])
            nc.sync.dma_start(cw1_t[:], CW1[:])
            nc.sync.dma_start(cw2_t[:], CW2[:])
            nc.sync.dma_start(cw3_t[:], CW3[:])
            nc.sync.dma_start(wfi2_t[:], Wfi2[:])
            nc.sync.dma_start(wfi3_t[:], Wfi3[:])
            nc.sync.dma_start(wma_t[:], Wm[0:128, :])
            nc.sync.dma_start(wmb_t[:], Wm[128:256, :])
            nc.sync.dma_start(wl_t[:], Wl[:])
            nc.sync.dma_start(bmT_t[:], bmT[:])
            nc.sync.dma_start(blT_t[:], blT[:])
            make_identity(nc, ident[:])
            nc.vector.memset(onec3[:], -1.0)  # for -|q|^2 row sums
            nc.gpsimd.load_library(library_config.mlp)
            last_rd = {}
            last_rW = {}

            # per-batch state tiles; t loop outer so the two batch chains interleave
            q5_b, q4_b, key5_b, fALL_b, aux_b = [], [], [], [], []
            for b in range(BPC):
                q5_b.append([sb.tile([5, N], fp32, tag=f"q5_{b}_{i}", name=f"q5_{b}_{i}") for i in range(2)])
                q4_b.append([sb.tile([4, N], bf16, tag=f"q4_{b}_{i}", name=f"q4_{b}_{i}") for i in range(2)])
                key5_b.append(sb.tile([5, N], fp32, tag=f"key5_{b}", name=f"key5_{b}"))
                fALL_b.append(sb.tile([128, 4, N], bf16, tag=f"fALL_{b}", name=f"fALL_{b}"))
                nc.vector.memset(fALL_b[b][:], 0.0)

            def qidx(t):
                return t % 2 if t < HALF else (t + 1) % 2

            for t in range(SEQ):
                for b in range(BPC):
                    q5, q4 = q5_b[b], q4_b[b]
                    key5, fALL = key5_b[b], fALL_b[b]
                    tab = tabs[b][t % 2]
                    qi = qidx(t)
                    qt5, qt4 = q5[qi], q4[qi]
                    if t < HALF:
                        base = b * SEQ + t
                        nc.sync.dma_start(qt5[:], q5_l[base, :, :])
                        nc.sync.dma_start(qt4[:], q4_l[base, :, :])
                        kbase = b * SEQ + max(t - 1, 0)
                        nc.sync.dma_start(key5[:], k5_l[kbase, :, :])
                        kt4 = q4[max(t - 1, 0) % 2]
                    elif t == HALF:
                        # q stays = frame5 tiles; keys = frame5 too
                        nc.sync.dma_start(key5[:], k5_l[b * SEQ + HALF - 1, :, :])
                        kt4 = q4[(HALF - 1) % 2]
                    else:
                        # key tiles derived at the end of step t-1
                        kt4 = q4[qidx(t - 1)]

                    # ---- A table: A[key] = [k;1]@WB + f@Wnf, per key chunk ----
                    wr_insts = []
                    for j in range(NCHUNK):
                        jj = slice(j * 128, (j + 1) * 128)
                        a_ps = ps.tile([128, CTOT], fp32, tag="a_ps", bufs=2)
                        nc.tensor.matmul(
                            a_ps[:, 0:64], kt4[:, jj], wb1_t[:], start=True, stop=False
                        )
                        nc.tensor.matmul(
                            a_ps[:, 0:64], fALL[0:64, 0, jj], wnf1_t[:],
                            start=False, stop=True,
                        )
                        nc.tensor.matmul(
                            a_ps[:, 64:192], kt4[:, jj], wb2_t[:], start=True, stop=False
                        )
                        nc.tensor.matmul(
                            a_ps[:, 64:192], fALL[:, 1, jj], wnf2_t[:],
                            start=False, stop=True,
                        )
                        nc.tensor.matmul(
                            a_ps[:, 192:448], kt4[:, jj], wb3_t[:], start=True, stop=False
                        )
                        nc.tensor.matmul(
                            a_ps[:, 192:448], fALL[:, 2, jj], wnf3a_t[:],
                            start=False, stop=False,
                        )
                        nc.tensor.matmul(
                            a_ps[:, 192:448], fALL[:, 3, jj], wnf3b_t[:],
                            start=False, stop=True,
                        )
                        a_sb = sb.tile([128, CTOT], bf16, tag="a_sb", bufs=4)
                        nc.scalar.copy(a_sb[:], a_ps[:])
                        w = nc.sync.dma_start(tab[jj, 0:448], a_sb[:])
                        wr_insts.append(w.ins)

                    # ---- d2 + top-8 per chunk; ordinal o = c*8+k (chunk-outer);
                    # per 2-chunk group: rewrap indices + dma_gather ----
                    Dt = Dts[b][t % 2]
                    Wt = Wts[b][t % 2]
                    idxu = sb.tile([128, NCHUNK, K], u16, tag=f"idxu_{b}", bufs=2)
                    addA = sb.tile([128, NCHUNK, K], bf16, tag=f"addA_{b}", bufs=2)
                    dst = sb.tile([128, 32, 512], bf16, tag=f"dst_{b}", bufs=1)
                    gather_insts = []
                    for j in range(NCHUNK):
                        jj = slice(j * 128, (j + 1) * 128)
                        d2_ps = ps.tile([128, N], fp32, tag="d2_ps", bufs=3)
                        nc.tensor.matmul(
                            d2_ps[:], qt5[:, jj], key5[:], start=True, stop=True
                        )
                        d2h = sb.tile([128, N], fp16, tag="d2h", bufs=4)
                        nc.scalar.copy(d2h[:], d2_ps[:])
                        vals = sb.tile([128, K], fp16, tag="vals", bufs=4)
                        idx8 = sb.tile([128, K], u16, tag="idx8", bufs=4)
                        nc.vector.max(vals[:], d2h[:])
                        nc.vector.max_index(idx8[:], vals[:], d2h[:])
                        nc.vector.tensor_copy(idxu[:, j, :], idx8[:])
                        # addend = (vals < -r^2) * -BIG  (0 for valid)
                        nc.vector.tensor_scalar(
                            addA[:, j, :], vals[:], -R1SQ, -BIG,
                            op0=mybir.AluOpType.is_lt, op1=mybir.AluOpType.mult,
                        )
                    # full-step index rewrap (validated path), then 2 group gathers
                    wD = nc.sync.dma_start(Dt[:, :], idxu[:])
                    for r in last_rd.get((b, t % 2), []):
                        add_dep_helper(wD.ins, r, reason="D reuse after prior read")
                    w16 = sb.tile([16, 32, 8], u16, tag=f"w16_{b}", bufs=2)
                    rd = nc.sync.dma_start(
                        w16[:], Dt[:, :].rearrange("(ph pp) o -> pp o ph", pp=16)
                    )
                    add_dep_helper(rd.ins, wD.ins, reason="read D after write")
                    last_rd[(b, t % 2)] = [rd.ins]
                    wW = nc.sync.dma_start(Wt[:, :], w16[:])
                    for r in last_rW.get((b, t % 2), []):
                        add_dep_helper(wW.ins, r, reason="W reuse after prior read")
                    w128 = sb.tile([128, 256], u16, tag=f"w128_{b}", bufs=2)
                    rW = nc.sync.dma_start(
                        w128[:],
                        Wt[:, :].unsqueeze(0).broadcast_to((8, 16, 256)),
                    )
                    add_dep_helper(rW.ins, wW.ins, reason="read W after write")
                    last_rW[(b, t % 2)] = [rW.ins]
                    for gg in range(2):
                        go = slice(gg * 16, (gg + 1) * 16)
                        gi = nc.gpsimd.dma_gather(
                            out_ap=dst[:, go, :],
                            in_ap=tab[:],
                            idxs_ap=w128[:, gg * 128 : (gg + 1) * 128].bitcast(
                                mybir.dt.int16
                            ),
                            num_idxs=16 * 128,
                            num_idxs_reg=16 * 128,
                            elem_size=512,
                            single_packet=False,
                        )
                        for w in wr_insts:
                            add_dep_helper(gi.ins, w, reason="gather after tab write")
                        gather_insts.append(gi)

                    # per-chunk max-pools (rows o = c*8 + k)
                    mAll1 = sb.tile([128, 4, 64], fp32, tag=f"mAll1_{b}", bufs=2)
                    mAll23 = sb.tile([128, 4, 384], fp32, tag=f"mAll23_{b}", bufs=2)
                    for j in range(NCHUNK):
                        j8 = j * 8
                        g1m = sb.tile([128, 8, 64], bf16, tag="g1m", bufs=2)
                        nc.vector.tensor_tensor(
                            g1m[:], dst[:, j8 : j8 + 8, 0:64],
                            addA[:, j, :].unsqueeze(2).broadcast_to((128, 8, 64)),
                            op=mybir.AluOpType.add,
                        )
                        p1a = sb.tile([128, 4, 64], bf16, tag="p1a", bufs=2)
                        nc.vector.tensor_tensor(
                            p1a[:], g1m[:, 0:4, :], g1m[:, 4:8, :],
                            op=mybir.AluOpType.max,
                        )
                        p1b = sb.tile([128, 2, 64], bf16, tag="p1b", bufs=2)
                        nc.vector.tensor_tensor(
                            p1b[:], p1a[:, 0:2, :], p1a[:, 2:4, :],
                            op=mybir.AluOpType.max,
                        )
                        p1c = sb.tile([128, 64], bf16, tag="p1c", bufs=2)
                        nc.vector.tensor_tensor(
                            p1c[:], p1b[:, 0, :], p1b[:, 1, :],
                            op=mybir.AluOpType.max,
                        )
                        nc.vector.tensor_tensor(
                            mAll1[:, j, :], p1c[:], dst[:, j8, 0:64],
                            op=mybir.AluOpType.max,
                        )
                        p2a = sb.tile([128, 4, 384], bf16, tag="p2a", bufs=2)
                        nc.vector.tensor_tensor(
                            p2a[:], dst[:, j8 : j8 + 4, 64:448],
                            dst[:, j8 + 4 : j8 + 8, 64:448],
                            op=mybir.AluOpType.max,
                        )
                        p2b = sb.tile([128, 2, 384], bf16, tag="p2b", bufs=2)
                        nc.vector.tensor_tensor(
                            p2b[:], p2a[:, 0:2, :], p2a[:, 2:4, :],
                            op=mybir.AluOpType.max,
                        )
                        nc.vector.tensor_tensor(
                            mAll23[:, j, :], p2b[:, 0, :], p2b[:, 1, :],
                            op=mybir.AluOpType.max,
                        )

                    # ---- transposed C + m^T accumulate, per cell, per chunk ----
                    for j in range(NCHUNK):
                        jj = slice(j * 128, (j + 1) * 128)
                        ct = ps.tile([128, 4, 128], fp32, tag="ct", bufs=2)
                        # cell1
                        nc.tensor.matmul(
                            ct[0:64, 0, :], cw1_t[:], qt4[0:3, jj], start=True, stop=False
                        )
                        nc.tensor.matmul(
                            ct[0:64, 0, :], mAll1[:, j, :], ident[:],
                            is_transpose=True, start=False, stop=True,
                        )
                        nc.scalar.copy(fALL[0:64, 0, jj], ct[0:64, 0, :])
                        # cell2
                        nc.tensor.matmul(
                            ct[:, 1, :], cw2_t[:], qt4[0:3, jj], start=True, stop=False
                        )
                        nc.tensor.matmul(
                            ct[:, 1, :], wfi2_t[:], fALL[0:64, 0, jj],
                            start=False, stop=False,
                        )
                        nc.tensor.matmul(
                            ct[:, 1, :], mAll23[:, j, 0:128], ident[:],
                            is_transpose=True, start=False, stop=True,
                        )
                        nc.scalar.copy(fALL[:, 1, jj], ct[:, 1, :])
                        # cell3
                        nc.tensor.matmul(
                            ct[:, 2, :], cw3_t[:, 0:128], qt4[0:3, jj],
                            start=True, stop=False,
                        )
                        nc.tensor.matmul(
                            ct[:, 2, :], wfi3_t[:, 0:128], fALL[:, 1, jj],
                            start=False, stop=False,
                        )
                        nc.tensor.matmul(
                            ct[:, 2, :], mAll23[:, j, 128:256], ident[:],
                            is_transpose=True, start=False, stop=True,
                        )
                        nc.tensor.matmul(
                            ct[:, 3, :], cw3_t[:, 128:256], qt4[0:3, jj],
                            start=True, stop=False,
                        )
                        nc.tensor.matmul(
                            ct[:, 3, :], wfi3_t[:, 128:256], fALL[:, 1, jj],
                            start=False, stop=False,
                        )
                        nc.tensor.matmul(
                            ct[:, 3, :], mAll23[:, j, 256:384], ident[:],
                            is_transpose=True, start=False, stop=True,
                        )
                        nc.scalar.copy(fALL[:, 2:4, jj], ct[:, 2:4, :])

                    if t >= HALF:
                        # motion MLP from f3 (fALL slices 2:4)
                        aux = ps.tile([128, N], fp32, tag="aux", bufs=1)
                        nc.tensor.matmul(
                            aux[0:H, :], wma_t[:], fALL[:, 2, :], start=True, stop=False
                        )
                        nc.tensor.matmul(
                            aux[0:H, :], wmb_t[:], fALL[:, 3, :], start=False, stop=True
                        )
                        hm_sb = sb.tile([H, N], bf16, tag=f"hm_sb_{b}")
                        nc.scalar.activation(
                            hm_sb[:], aux[0:H, :],
                            mybir.ActivationFunctionType.Relu,
                            bias=bmT_t[:], scale=1.0,
                        )
                        nc.tensor.matmul(
                            aux[0:3, :], wl_t[:], hm_sb[:], start=True, stop=True
                        )
                        # next query = q + motion + bl
                        nq5 = q5[qidx(t + 1) if t + 1 < SEQ else (t % 2)]
                        nq4 = q4[qidx(t + 1) if t + 1 < SEQ else (t % 2)]
                        nc.vector.scalar_tensor_tensor(
                            nq5[0:3, :], aux[0:3, :], blT_t[:], qt5[0:3, :],
                            op0=mybir.AluOpType.add, op1=mybir.AluOpType.add,
                        )
                        row = (b * HALF + (t - HALF)) * 3
                        nc.sync.dma_start(preds[row : row + 3, :], nq5[0:3, :])
                        if t + 1 < SEQ:
                            # derive next-step q aux rows and key tiles
                            sq3 = sb.tile([3, N], fp32, tag=f"sq3_{b}", bufs=2)
                            nc.vector.tensor_tensor(
                                sq3[:], nq5[0:3, :], nq5[0:3, :],
                                op=mybir.AluOpType.mult,
                            )
                            nc.tensor.matmul(
                                aux[0:1, :], onec3[:], sq3[:], start=True, stop=True
                            )
                            nqs_sb = sb.tile([1, N], fp32, tag="nqs_sb", bufs=2)
                            nc.scalar.copy(nqs_sb[:], aux[0:1, :])
                            nc.sync.dma_start(nq5[4:5, :], nqs_sb[:])
                            nc.scalar.copy(nq4[0:3, :], nq5[0:3, :])
                            # key tiles for t+1 = current q (scaled forms)
                            nc.vector.tensor_scalar(
                                key5[0:3, :], qt5[0:3, :], 2.0, None,
                                op0=mybir.AluOpType.mult,
                            )
                            nc.sync.dma_start(key5[3:4, :], qt5[4:5, :])

    nc.finalize()
    _CACHE["nc"] = nc
    return nc


def _prep_weights(inputs):
    W1, b1 = inputs["W1"], inputs["b1"]
    W2, b2 = inputs["W2"], inputs["b2"]
    W3, b3 = inputs["W3"], inputs["b3"]

    def wb(W, bvec, cout):
        return np.ascontiguousarray(
            np.concatenate([W[0:3], bvec[None, :]], axis=0), np.float32
        )


    return {
        "WB1": wb(W1, b1, 64),
        "WB2": wb(W2, b2, 128),
        "WB3": wb(W3, b3, 256),
        "Wnf1": np.ascontiguousarray(W1[3:67], np.float32),
        "Wnf2": np.ascontiguousarray(W2[67:195], np.float32),
        "Wnf3": np.ascontiguousarray(W3[131:387], np.float32),
        "CW1": np.ascontiguousarray(-W1[0:3], np.float32),
        "CW2": np.ascontiguousarray(-W2[0:3], np.float32),
        "CW3": np.ascontiguousarray(-W3[0:3], np.float32),
        "Wfi2": np.ascontiguousarray(W2[3:67], np.float32),
        "Wfi3": np.ascontiguousarray(W3[3:131], np.float32),
        "Wm": np.ascontiguousarray(inputs["Wm"], np.float32),
        "Wl": np.ascontiguousarray(inputs["Wl"], np.float32),
        "bmT": np.ascontiguousarray(inputs["bm"][:, None], np.float32),
        "blT": np.ascontiguousarray(inputs["bl"][:, None], np.float32),
    }


def _to_bf16(x):
    import ml_dtypes

    return np.asarray(x, np.float32).astype(ml_dtypes.bfloat16)


def _prep_frames(frames):
    # frames (BPC, SEQ, N, 3) for one core -> q5/k5 fp32 and q4 bf16 rows
    x = frames.transpose(0, 1, 3, 2)  # (BPC, SEQ, 3, N)
    ssq = np.sum(x * x, axis=2, keepdims=True)  # (BPC, SEQ, 1, N)
    ones = np.ones_like(ssq)
    q5 = np.concatenate([x, ones, -ssq], axis=2).reshape(BPC * SEQ, 5, N)
    k5 = np.concatenate([2.0 * x, -ssq, ones], axis=2).reshape(BPC * SEQ, 5, N)
    q4 = np.concatenate([x, ones], axis=2).reshape(BPC * SEQ, 4, N)
    return (
        np.ascontiguousarray(q5, np.float32),
        np.ascontiguousarray(k5, np.float32),
        _to_bf16(np.ascontiguousarray(q4, np.float32)),
    )


def _run(inputs, **spmd_kwargs):
    nc = _build()
    frames = np.asarray(inputs["frames"], np.float32)  # (16,12,512,3)
    wd = _prep_weights(
        {k: np.asarray(v, np.float32) for k, v in inputs.items() if k != "frames"}
    )
    bf_names = {
        "WB1", "WB2", "WB3", "Wnf1", "Wnf2", "Wnf3", "CW1", "CW2", "CW3",
        "Wfi2", "Wfi3", "Wm", "Wl",
    }
    wd = {k: (_to_bf16(v) if k in bf_names else v) for k, v in wd.items()}
    in_maps = []
    for c in range(NCORES):
        d = dict(wd)
        q5c, k5c, q4c = _prep_frames(frames[c * BPC : (c + 1) * BPC])
        d["q5_l"] = q5c
        d["k5_l"] = k5c
        d["q4_l"] = q4c
        in_maps.append(d)
    res = run_bass_kernel_spmd(nc, in_maps, list(range(NCORES)), **spmd_kwargs)
    outs = []
    for c in range(NCORES):
        p = np.asarray(res.results[c]["preds"]).reshape(BPC, HALF, 3, N)
        outs.append(p.transpose(0, 1, 3, 2))  # (2,6,512,3)
    return np.ascontiguousarray(np.concatenate(outs, axis=0), np.float32), res


def kernel(**inputs):
    return _run(inputs)[0]


# revision 27
# speedup vs baseline: 1.3264x; 1.0646x over previous
import sys

sys.path.insert(0, "/opt/trn_rl_repo")
import numpy as np
from concourse import bass, bacc, tile, mybir
from concourse.bass_utils import run_bass_kernel_spmd
from concourse.masks import make_identity
from concourse import library_config
from concourse.tile import add_dep_helper

fp32 = mybir.dt.float32
bf16 = mybir.dt.bfloat16
u32 = mybir.dt.uint32
u16 = mybir.dt.uint16
fp16 = mybir.dt.float16
u8 = mybir.dt.uint8

SEQ = 12
HALF = 6
N = 512
K = 8
NCHUNK = 4  # 512 queries / 128
BPC = 2  # batches per core
NCORES = 8
CTOT = 448  # 64+128+256
R1SQ = float(np.float32(4.0 + 1e-6) * np.float32(4.0 + 1e-6))
H = 64  # motion MLP hidden
BIG = 3.0e4  # -BIG marks out-of-radius neighbors before the max

_CACHE = {}


def _build():
    if "nc" in _CACHE:
        return _CACHE["nc"]
    nc = bacc.Bacc(target_bir_lowering=False)

    # host-precomputed per-frame transposed tensors
    q5_l = nc.dram_tensor("q5_l", (BPC * SEQ, 5, N), fp32, kind="ExternalInput")
    k5_l = nc.dram_tensor("k5_l", (BPC * SEQ, 5, N), fp32, kind="ExternalInput")
    q4_l = nc.dram_tensor("q4_l", (BPC * SEQ, 4, N), bf16, kind="ExternalInput")
    # weights (bf16 feature path)
    WB1 = nc.dram_tensor("WB1", (4, 64), bf16, kind="ExternalInput")
    WB2 = nc.dram_tensor("WB2", (4, 128), bf16, kind="ExternalInput")
    WB3 = nc.dram_tensor("WB3", (4, 256), bf16, kind="ExternalInput")
    Wnf1 = nc.dram_tensor("Wnf1", (64, 64), bf16, kind="ExternalInput")
    Wnf2 = nc.dram_tensor("Wnf2", (128, 128), bf16, kind="ExternalInput")
    Wnf3 = nc.dram_tensor("Wnf3", (256, 256), bf16, kind="ExternalInput")
    CW1 = nc.dram_tensor("CW1", (3, 64), bf16, kind="ExternalInput")
    CW2 = nc.dram_tensor("CW2", (3, 128), bf16, kind="ExternalInput")
    CW3 = nc.dram_tensor("CW3", (3, 256), bf16, kind="ExternalInput")
    Wfi2 = nc.dram_tensor("Wfi2", (64, 128), bf16, kind="ExternalInput")
    Wfi3 = nc.dram_tensor("Wfi3", (128, 256), bf16, kind="ExternalInput")
    Wm = nc.dram_tensor("Wm", (256, H), bf16, kind="ExternalInput")
    Wl = nc.dram_tensor("Wl", (H, 3), bf16, kind="ExternalInput")
    bmT = nc.dram_tensor("bmT", (H, 1), fp32, kind="ExternalInput")
    blT = nc.dram_tensor("blT", (3, 1), fp32, kind="ExternalInput")

    preds = nc.dram_tensor("preds", (BPC * HALF * 3, N), fp32, kind="ExternalOutput")
    tabs = [
        [nc.dram_tensor(f"tab_b{b}_p{p}", (N, 512), bf16) for p in range(2)]
        for b in range(BPC)
    ]
    Dts = [
        [nc.dram_tensor(f"D_b{b}_p{p}", (128, 32), u16) for p in range(2)]
        for b in range(BPC)
    ]
    Wts = [
        [nc.dram_tensor(f"W_b{b}_p{p}", (16, 256), u16) for p in range(2)]
        for b in range(BPC)
    ]

    with tile.TileContext(nc) as tc:
        with tc.tile_pool(name="sb", bufs=1) as sb, tc.tile_pool(
            name="ps", bufs=1, space="PSUM"
        ) as ps:
            # ---- persistent weights in SBUF ----
            wb1_t = sb.tile([4, 64], bf16, tag="wb1")
            wb2_t = sb.tile([4, 128], bf16, tag="wb2")
            wb3_t = sb.tile([4, 256], bf16, tag="wb3")
            wnf1_t = sb.tile([64, 64], bf16, tag="wnf1")
            wnf2_t = sb.tile([128, 128], bf16, tag="wnf2")
            wnf3a_t = sb.tile([128, 256], bf16, tag="wnf3a")
            wnf3b_t = sb.tile([128, 256], bf16, tag="wnf3b")
            cw1_t = sb.tile([3, 64], bf16, tag="cw1")
            cw2_t = sb.tile([3, 128], bf16, tag="cw2")
            cw3_t = sb.tile([3, 256], bf16, tag="cw3")
            wfi2_t = sb.tile([64, 128], bf16, tag="wfi2")
            wfi3_t = sb.tile([128, 256], bf16, tag="wfi3")
            wma_t = sb.tile([128, H], bf16, tag="wma")
            wmb_t = sb.tile([128, H], bf16, tag="wmb")
            wl_t = sb.tile([H, 3], bf16, tag="wl")
            bmT_t = sb.tile([H, 1], fp32, tag="bmT")
            blT_t = sb.tile([3, 1], fp32, tag="blT")
            ident = sb.tile([128, 128], fp32, tag="ident")
            onec3 = sb.tile([3, 1], fp32, tag="onec3")
            nc.sync.dma_start(wb1_t[:], WB1[:])
            nc.sync.dma_start(wb2_t[:], WB2[:])
            nc.sync.dma_start(wb3_t[:], WB3[:])
            nc.sync.dma_start(wnf1_t[:], Wnf1[:])
            nc.sync.dma_start(wnf2_t[:], Wnf2[:])
            nc.sync.dma_start(wnf3a_t[:], Wnf3[0:128, :])
            nc.sync.dma_start(wnf3b_t[:], Wnf3[128:256, :])
            nc.sync.dma_start(cw1_t[:], CW1[:])
            nc.sync.dma_start(cw2_t[:], CW2[:])
            nc.sync.dma_start(cw3_t[:], CW3[:])
            nc.sync.dma_start(wfi2_t[:], Wfi2[:])
            nc.sync.dma_start(wfi3_t[:], Wfi3[:])
            nc.sync.dma_start(wma_t[:], Wm[0:128, :])
            nc.sync.dma_start(wmb_t[:], Wm[128:256, :])
            nc.sync.dma_start(wl_t[:], Wl[:])
            nc.sync.dma_start(bmT_t[:], bmT[:])
            nc.sync.dma_start(blT_t[:], blT[:])
            make_identity(nc, ident[:])
            nc.vector.memset(onec3[:], -1.0)  # for -|q|^2 row sums
            nc.gpsimd.load_library(library_config.mlp)
            last_rd = {}
            last_rW = {}

            # per-batch state tiles; t loop outer so the two batch chains interleave
            q5_b, q4_b, key5_b, fALL_b, aux_b = [], [], [], [], []
            for b in range(BPC):
                q5_b.append([sb.tile([5, N], fp32, tag=f"q5_{b}_{i}", name=f"q5_{b}_{i}") for i in range(2)])
                q4_b.append([sb.tile([4, N], bf16, tag=f"q4_{b}_{i}", name=f"q4_{b}_{i}") for i in range(2)])
                key5_b.append(sb.tile([5, N], fp32, tag=f"key5_{b}", name=f"key5_{b}"))
                fALL_b.append(sb.tile([128, 4, N], bf16, tag=f"fALL_{b}", name=f"fALL_{b}"))
                nc.vector.memset(fALL_b[b][:], 0.0)

            def qidx(t):
                return t % 2 if t < HALF else (t + 1) % 2

            for t in range(SEQ):
                for b in range(BPC):
                    q5, q4 = q5_b[b], q4_b[b]
                    key5, fALL = key5_b[b], fALL_b[b]
                    tab = tabs[b][t % 2]
                    qi = qidx(t)
                    qt5, qt4 = q5[qi], q4[qi]
                    if t < HALF:
                        base = b * SEQ + t
                        nc.sync.dma_start(qt5[:], q5_l[base, :, :])
                        nc.sync.dma_start(qt4[:], q4_l[base, :, :])
                        kbase = b * SEQ + max(t - 1, 0)
                        nc.sync.dma_start(key5[:], k5_l[kbase, :, :])
                        kt4 = q4[max(t - 1, 0) % 2]
                    elif t == HALF:
                        # q stays = frame5 tiles; keys = frame5 too
                        nc.sync.dma_start(key5[:], k5_l[b * SEQ + HALF - 1, :, :])
                        kt4 = q4[(HALF - 1) % 2]
                    else:
                        # key tiles derived at the end of step t-1
                        kt4 = q4[qidx(t - 1)]

                    # ---- A table: A[key] = [k;1]@WB + f@Wnf, per key chunk ----
                    wr_insts = []
                    for j in range(NCHUNK):
                        jj = slice(j * 128, (j + 1) * 128)
                        a_ps = ps.tile([128, CTOT], fp32, tag="a_ps", bufs=2)
                        nc.tensor.matmul(
                            a_ps[:, 0:64], kt4[:, jj], wb1_t[:], start=True, stop=False
                        )
                        nc.tensor.matmul(
                            a_ps[:, 0:64], fALL[0:64, 0, jj], wnf1_t[:],
                            start=False, stop=True,
                        )
                        nc.tensor.matmul(
                            a_ps[:, 64:192], kt4[:, jj], wb2_t[:], start=True, stop=False
                        )
                        nc.tensor.matmul(
                            a_ps[:, 64:192], fALL[:, 1, jj], wnf2_t[:],
                            start=False, stop=True,
                        )
                        nc.tensor.matmul(
                            a_ps[:, 192:448], kt4[:, jj], wb3_t[:], start=True, stop=False
                        )
                        nc.tensor.matmul(
                            a_ps[:, 192:448], fALL[:, 2, jj], wnf3a_t[:],
                            start=False, stop=False,
                        )
                        nc.tensor.matmul(
                            a_ps[:, 192:448], fALL[:, 3, jj], wnf3b_t[:],
                            start=False, stop=True,
                        )
                        a_sb = sb.tile([128, CTOT], bf16, tag="a_sb", bufs=4)
                        nc.scalar.copy(a_sb[:], a_ps[:])
                        w = nc.sync.dma_start(tab[jj, 0:448], a_sb[:])
                        wr_insts.append(w.ins)

                    # ---- d2 + top-8 per chunk; ordinal o = c*8+k (chunk-outer);
                    # per 2-chunk group: rewrap indices + dma_gather ----
                    Dt = Dts[b][t % 2]
                    Wt = Wts[b][t % 2]
                    idxu = sb.tile([128, NCHUNK, K], u16, tag=f"idxu_{b}", bufs=2)
                    addA = sb.tile([128, NCHUNK, K], bf16, tag=f"addA_{b}", bufs=2)
                    dst = sb.tile([128, 32, 512], bf16, tag=f"dst_{b}", bufs=1)
                    gather_insts = []
                    for j in range(NCHUNK):
                        jj = slice(j * 128, (j + 1) * 128)
                        d2_ps = ps.tile([128, N], fp32, tag="d2_ps", bufs=3)
                        nc.tensor.matmul(
                            d2_ps[:], qt5[:, jj], key5[:], start=True, stop=True
                        )
                        d2h = sb.tile([128, N], fp16, tag="d2h", bufs=4)
                        nc.scalar.copy(d2h[:], d2_ps[:])
                        vals = sb.tile([128, K], fp16, tag="vals", bufs=4)
                        idx8 = sb.tile([128, K], u16, tag="idx8", bufs=4)
                        nc.vector.max(vals[:], d2h[:])
                        nc.vector.max_index(idx8[:], vals[:], d2h[:])
                        nc.vector.tensor_copy(idxu[:, j, :], idx8[:])
                        # addend = (vals < -r^2) * -BIG  (0 for valid)
                        nc.vector.tensor_scalar(
                            addA[:, j, :], vals[:], -R1SQ, -BIG,
                            op0=mybir.AluOpType.is_lt, op1=mybir.AluOpType.mult,
                        )
                    # full-step index rewrap (validated path), then 2 group gathers
                    wD = nc.sync.dma_start(Dt[:, :], idxu[:])
                    for r in last_rd.get((b, t % 2), []):
                        add_dep_helper(wD.ins, r, reason="D reuse after prior read")
                    w16 = sb.tile([16, 32, 8], u16, tag=f"w16_{b}", bufs=2)
                    rd = nc.sync.dma_start(
                        w16[:], Dt[:, :].rearrange("(ph pp) o -> pp o ph", pp=16)
                    )
                    add_dep_helper(rd.ins, wD.ins, reason="read D after write")
                    last_rd[(b, t % 2)] = [rd.ins]
                    wW = nc.sync.dma_start(Wt[:, :], w16[:])
                    for r in last_rW.get((b, t % 2), []):
                        add_dep_helper(wW.ins, r, reason="W reuse after prior read")
                    w128 = sb.tile([128, 256], u16, tag=f"w128_{b}", bufs=2)
                    rW = nc.sync.dma_start(
                        w128[:],
                        Wt[:, :].unsqueeze(0).broadcast_to((8, 16, 256)),
                    )
                    add_dep_helper(rW.ins, wW.ins, reason="read W after write")
                    last_rW[(b, t % 2)] = [rW.ins]
                    for cc in range(NCHUNK):
                        co = slice(cc * 8, (cc + 1) * 8)
                        gi = nc.gpsimd.dma_gather(
                            out_ap=dst[:, co, :],
                            in_ap=tab[:],
                            idxs_ap=w128[:, cc * 64 : (cc + 1) * 64].bitcast(
                                mybir.dt.int16
                            ),
                            num_idxs=8 * 128,
                            num_idxs_reg=8 * 128,
                            elem_size=512,
                            single_packet=False,
                        )
                        for w in wr_insts:
                            add_dep_helper(gi.ins, w, reason="gather after tab write")
                        gather_insts.append(gi)

                    # per-chunk max-pools (rows o = c*8 + k)
                    mAll1 = sb.tile([128, 4, 64], fp32, tag=f"mAll1_{b}", bufs=2)
                    mAll23 = sb.tile([128, 4, 384], fp32, tag=f"mAll23_{b}", bufs=2)
                    for j in range(NCHUNK):
                        j8 = j * 8
                        g1m = sb.tile([128, 8, 64], bf16, tag="g1m", bufs=2)
                        nc.vector.tensor_tensor(
                            g1m[:], dst[:, j8 : j8 + 8, 0:64],
                            addA[:, j, :].unsqueeze(2).broadcast_to((128, 8, 64)),
                            op=mybir.AluOpType.add,
                        )
                        p1a = sb.tile([128, 4, 64], bf16, tag="p1a", bufs=2)
                        nc.vector.tensor_tensor(
                            p1a[:], g1m[:, 0:4, :], g1m[:, 4:8, :],
                            op=mybir.AluOpType.max,
                        )
                        p1b = sb.tile([128, 2, 64], bf16, tag="p1b", bufs=2)
                        nc.vector.tensor_tensor(
                            p1b[:], p1a[:, 0:2, :], p1a[:, 2:4, :],
                            op=mybir.AluOpType.max,
                        )
                        p1c = sb.tile([128, 64], bf16, tag="p1c", bufs=2)
                        nc.vector.tensor_tensor(
                            p1c[:], p1b[:, 0, :], p1b[:, 1, :],
                            op=mybir.AluOpType.max,
                        )
                        nc.vector.tensor_tensor(
                            mAll1[:, j, :], p1c[:], dst[:, j8, 0:64],
                            op=mybir.AluOpType.max,
                        )
                        p2a = sb.tile([128, 4, 384], bf16, tag="p2a", bufs=2)
                        nc.vector.tensor_tensor(
                            p2a[:], dst[:, j8 : j8 + 4, 64:448],
                            dst[:, j8 + 4 : j8 + 8, 64:448],
                            op=mybir.AluOpType.max,
                        )
                        p2b = sb.tile([128, 2, 384], bf16, tag="p2b", bufs=2)
                        nc.vector.tensor_tensor(
                            p2b[:], p2a[:, 0:2, :], p2a[:, 2:4, :],
                            op=mybir.AluOpType.max,
                        )
                        nc.vector.tensor_tensor(
                            mAll23[:, j, :], p2b[:, 0, :], p2b[:, 1, :],
                            op=mybir.AluOpType.max,
                        )

                    # ---- transposed C + m^T accumulate, per cell, per chunk ----
                    for j in range(NCHUNK):
                        jj = slice(j * 128, (j + 1) * 128)
                        ct = ps.tile([128, 4, 128], fp32, tag="ct", bufs=2)
                        # cell1
                        nc.tensor.matmul(
                            ct[0:64, 0, :], cw1_t[:], qt4[0:3, jj], start=True, stop=False
                        )
                        nc.tensor.matmul(
                            ct[0:64, 0, :], mAll1[:, j, :], ident[:],
                            is_transpose=True, start=False, stop=True,
                        )
                        nc.scalar.copy(fALL[0:64, 0, jj], ct[0:64, 0, :])
                        # cell2
                        nc.tensor.matmul(
                            ct[:, 1, :], cw2_t[:], qt4[0:3, jj], start=True, stop=False
                        )
                        nc.tensor.matmul(
                            ct[:, 1, :], wfi2_t[:], fALL[0:64, 0, jj],
                            start=False, stop=False,
                        )
                        nc.tensor.matmul(
                            ct[:, 1, :], mAll23[:, j, 0:128], ident[:],
                            is_transpose=True, start=False, stop=True,
                        )
                        nc.scalar.copy(fALL[:, 1, jj], ct[:, 1, :])
                        # cell3
                        nc.tensor.matmul(
                            ct[:, 2, :], cw3_t[:, 0:128], qt4[0:3, jj],
                            start=True, stop=False,
                        )
                        nc.tensor.matmul(
                            ct[:, 2, :], wfi3_t[:, 0:128], fALL[:, 1, jj],
                            start=False, stop=False,
                        )
                        nc.tensor.matmul(
                            ct[:, 2, :], mAll23[:, j, 128:256], ident[:],
                            is_transpose=True, start=False, stop=True,
                        )
                        nc.tensor.matmul(
                            ct[:, 3, :], cw3_t[:, 128:256], qt4[0:3, jj],
                            start=True, stop=False,
                        )
                        nc.tensor.matmul(
                            ct[:, 3, :], wfi3_t[:, 128:256], fALL[:, 1, jj],
                            start=False, stop=False,
                        )
                        nc.tensor.matmul(
                            ct[:, 3, :], mAll23[:, j, 256:384], ident[:],
                            is_transpose=True, start=False, stop=True,
                        )
                        nc.scalar.copy(fALL[:, 2:4, jj], ct[:, 2:4, :])

                    if t >= HALF:
                        # motion MLP from f3 (fALL slices 2:4)
                        aux = ps.tile([128, N], fp32, tag="aux", bufs=1)
                        nc.tensor.matmul(
                            aux[0:H, :], wma_t[:], fALL[:, 2, :], start=True, stop=False
                        )
                        nc.tensor.matmul(
                            aux[0:H, :], wmb_t[:], fALL[:, 3, :], start=False, stop=True
                        )
                        hm_sb = sb.tile([H, N], bf16, tag=f"hm_sb_{b}")
                        nc.scalar.activation(
                            hm_sb[:], aux[0:H, :],
                            mybir.ActivationFunctionType.Relu,
                            bias=bmT_t[:], scale=1.0,
                        )
                        nc.tensor.matmul(
                            aux[0:3, :], wl_t[:], hm_sb[:], start=True, stop=True
                        )
                        # next query = q + motion + bl
                        nq5 = q5[qidx(t + 1) if t + 1 < SEQ else (t % 2)]
                        nq4 = q4[qidx(t + 1) if t + 1 < SEQ else (t % 2)]
                        nc.vector.scalar_tensor_tensor(
                            nq5[0:3, :], aux[0:3, :], blT_t[:], qt5[0:3, :],
                            op0=mybir.AluOpType.add, op1=mybir.AluOpType.add,
                        )
                        row = (b * HALF + (t - HALF)) * 3
                        nc.sync.dma_start(preds[row : row + 3, :], nq5[0:3, :])
                        if t + 1 < SEQ:
                            # derive next-step q aux rows and key tiles
                            sq3 = sb.tile([3, N], fp32, tag=f"sq3_{b}", bufs=2)
                            nc.vector.tensor_tensor(
                                sq3[:], nq5[0:3, :], nq5[0:3, :],
                                op=mybir.AluOpType.mult,
                            )
                            nc.tensor.matmul(
                                aux[0:1, :], onec3[:], sq3[:], start=True, stop=True
                            )
                            nqs_sb = sb.tile([1, N], fp32, tag="nqs_sb", bufs=2)
                            nc.scalar.copy(nqs_sb[:], aux[0:1, :])
                            nc.sync.dma_start(nq5[4:5, :], nqs_sb[:])
                            nc.scalar.copy(nq4[0:3, :], nq5[0:3, :])
                            # key tiles for t+1 = current q (scaled forms)
                            nc.vector.tensor_scalar(
                                key5[0:3, :], qt5[0:3, :], 2.0, None,
                                op0=mybir.AluOpType.mult,
                            )
                            nc.sync.dma_start(key5[3:4, :], qt5[4:5, :])

    nc.finalize()
    _CACHE["nc"] = nc
    return nc


def _prep_weights(inputs):
    W1, b1 = inputs["W1"], inputs["b1"]
    W2, b2 = inputs["W2"], inputs["b2"]
    W3, b3 = inputs["W3"], inputs["b3"]

    def wb(W, bvec, cout):
        return np.ascontiguousarray(
            np.concatenate([W[0:3], bvec[None, :]], axis=0), np.float32
        )


    return {
        "WB1": wb(W1, b1, 64),
        "WB2": wb(W2, b2, 128),
        "WB3": wb(W3, b3, 256),
        "Wnf1": np.ascontiguousarray(W1[3:67], np.float32),
        "Wnf2": np.ascontiguousarray(W2[67:195], np.float32),
        "Wnf3": np.ascontiguousarray(W3[131:387], np.float32),
        "CW1": np.ascontiguousarray(-W1[0:3], np.float32),
        "CW2": np.ascontiguousarray(-W2[0:3], np.float32),
        "CW3": np.ascontiguousarray(-W3[0:3], np.float32),
        "Wfi2": np.ascontiguousarray(W2[3:67], np.float32),
        "Wfi3": np.ascontiguousarray(W3[3:131], np.float32),
        "Wm": np.ascontiguousarray(inputs["Wm"], np.float32),
        "Wl": np.ascontiguousarray(inputs["Wl"], np.float32),
        "bmT": np.ascontiguousarray(inputs["bm"][:, None], np.float32),
        "blT": np.ascontiguousarray(inputs["bl"][:, None], np.float32),
    }


def _to_bf16(x):
    import ml_dtypes

    return np.asarray(x, np.float32).astype(ml_dtypes.bfloat16)


def _prep_frames(frames):
    # frames (BPC, SEQ, N, 3) for one core -> q5/k5 fp32 and q4 bf16 rows
    x = frames.transpose(0, 1, 3, 2)  # (BPC, SEQ, 3, N)
    ssq = np.sum(x * x, axis=2, keepdims=True)  # (BPC, SEQ, 1, N)
    ones = np.ones_like(ssq)
    q5 = np.concatenate([x, ones, -ssq], axis=2).reshape(BPC * SEQ, 5, N)
    k5 = np.concatenate([2.0 * x, -ssq, ones], axis=2).reshape(BPC * SEQ, 5, N)
    q4 = np.concatenate([x, ones], axis=2).reshape(BPC * SEQ, 4, N)
    return (
        np.ascontiguousarray(q5, np.float32),
        np.ascontiguousarray(k5, np.float32),
        _to_bf16(np.ascontiguousarray(q4, np.float32)),
    )


def _run(inputs, **spmd_kwargs):
    nc = _build()
    frames = np.asarray(inputs["frames"], np.float32)  # (16,12,512,3)
    wd = _prep_weights(
        {k: np.asarray(v, np.float32) for k, v in inputs.items() if k != "frames"}
    )
    bf_names = {
        "WB1", "WB2", "WB3", "Wnf1", "Wnf2", "Wnf3", "CW1", "CW2", "CW3",
        "Wfi2", "Wfi3", "Wm", "Wl",
    }
    wd = {k: (_to_bf16(v) if k in bf_names else v) for k, v in wd.items()}
    in_maps = []
    for c in range(NCORES):
        d = dict(wd)
        q5c, k5c, q4c = _prep_frames(frames[c * BPC : (c + 1) * BPC])
        d["q5_l"] = q5c
        d["k5_l"] = k5c
        d["q4_l"] = q4c
        in_maps.append(d)
    res = run_bass_kernel_spmd(nc, in_maps, list(range(NCORES)), **spmd_kwargs)
    outs = []
    for c in range(NCORES):
        p = np.asarray(res.results[c]["preds"]).reshape(BPC, HALF, 3, N)
        outs.append(p.transpose(0, 1, 3, 2))  # (2,6,512,3)
    return np.ascontiguousarray(np.concatenate(outs, axis=0), np.float32), res


def kernel(**inputs):
    return _run(inputs)[0]


# revision 28
# speedup vs baseline: 1.3924x; 1.0497x over previous
import sys

sys.path.insert(0, "/opt/trn_rl_repo")
import numpy as np
from concourse import bass, bacc, tile, mybir
from concourse.bass_utils import run_bass_kernel_spmd
from concourse.masks import make_identity
from concourse import library_config
from concourse.tile import add_dep_helper

fp32 = mybir.dt.float32
bf16 = mybir.dt.bfloat16
u32 = mybir.dt.uint32
u16 = mybir.dt.uint16
fp16 = mybir.dt.float16
u8 = mybir.dt.uint8

SEQ = 12
HALF = 6
N = 512
K = 8
NCHUNK = 4  # 512 queries / 128
BPC = 2  # batches per core
NCORES = 8
CTOT = 448  # 64+128+256
R1SQ = float(np.float32(4.0 + 1e-6) * np.float32(4.0 + 1e-6))
H = 64  # motion MLP hidden
BIG = 3.0e4  # -BIG marks out-of-radius neighbors before the max

_CACHE = {}


def _build():
    if "nc" in _CACHE:
        return _CACHE["nc"]
    nc = bacc.Bacc(target_bir_lowering=False)

    # host-precomputed per-frame transposed tensors
    q5_l = nc.dram_tensor("q5_l", (BPC * SEQ, 5, N), fp32, kind="ExternalInput")
    k5_l = nc.dram_tensor("k5_l", (BPC * SEQ, 5, N), fp32, kind="ExternalInput")
    q4_l = nc.dram_tensor("q4_l", (BPC * SEQ, 4, N), bf16, kind="ExternalInput")
    # weights (bf16 feature path)
    WB1 = nc.dram_tensor("WB1", (4, 64), bf16, kind="ExternalInput")
    WB2 = nc.dram_tensor("WB2", (4, 128), bf16, kind="ExternalInput")
    WB3 = nc.dram_tensor("WB3", (4, 256), bf16, kind="ExternalInput")
    Wnf1 = nc.dram_tensor("Wnf1", (64, 64), bf16, kind="ExternalInput")
    Wnf2 = nc.dram_tensor("Wnf2", (128, 128), bf16, kind="ExternalInput")
    Wnf3 = nc.dram_tensor("Wnf3", (256, 256), bf16, kind="ExternalInput")
    CW1 = nc.dram_tensor("CW1", (3, 64), bf16, kind="ExternalInput")
    CW2 = nc.dram_tensor("CW2", (3, 128), bf16, kind="ExternalInput")
    CW3 = nc.dram_tensor("CW3", (3, 256), bf16, kind="ExternalInput")
    Wfi2 = nc.dram_tensor("Wfi2", (64, 128), bf16, kind="ExternalInput")
    Wfi3 = nc.dram_tensor("Wfi3", (128, 256), bf16, kind="ExternalInput")
    Wm = nc.dram_tensor("Wm", (256, H), bf16, kind="ExternalInput")
    Wl = nc.dram_tensor("Wl", (H, 3), bf16, kind="ExternalInput")
    bmT = nc.dram_tensor("bmT", (H, 1), fp32, kind="ExternalInput")
    blT = nc.dram_tensor("blT", (3, 1), fp32, kind="ExternalInput")

    preds = nc.dram_tensor("preds", (BPC * HALF * 3, N), fp32, kind="ExternalOutput")
    tabs = [
        [nc.dram_tensor(f"tab_b{b}_p{p}", (N, 512), bf16) for p in range(2)]
        for b in range(BPC)
    ]
    Dts = [
        [nc.dram_tensor(f"D_b{b}_p{p}", (128, 32), u16) for p in range(2)]
        for b in range(BPC)
    ]
    Wts = [
        [nc.dram_tensor(f"W_b{b}_p{p}", (16, 256), u16) for p in range(2)]
        for b in range(BPC)
    ]

    with tile.TileContext(nc) as tc:
        with tc.tile_pool(name="sb", bufs=1) as sb, tc.tile_pool(
            name="ps", bufs=1, space="PSUM"
        ) as ps:
            # ---- persistent weights in SBUF ----
            wb1_t = sb.tile([4, 64], bf16, tag="wb1")
            wb2_t = sb.tile([4, 128], bf16, tag="wb2")
            wb3_t = sb.tile([4, 256], bf16, tag="wb3")
            wnf1_t = sb.tile([64, 64], bf16, tag="wnf1")
            wnf2_t = sb.tile([128, 128], bf16, tag="wnf2")
            wnf3a_t = sb.tile([128, 256], bf16, tag="wnf3a")
            wnf3b_t = sb.tile([128, 256], bf16, tag="wnf3b")
            cw1_t = sb.tile([3, 64], bf16, tag="cw1")
            cw2_t = sb.tile([3, 128], bf16, tag="cw2")
            cw3_t = sb.tile([3, 256], bf16, tag="cw3")
            wfi2_t = sb.tile([64, 128], bf16, tag="wfi2")
            wfi3_t = sb.tile([128, 256], bf16, tag="wfi3")
            wma_t = sb.tile([128, H], bf16, tag="wma")
            wmb_t = sb.tile([128, H], bf16, tag="wmb")
            wl_t = sb.tile([H, 3], bf16, tag="wl")
            bmT_t = sb.tile([H, 1], fp32, tag="bmT")
            blT_t = sb.tile([3, 1], fp32, tag="blT")
            ident = sb.tile([128, 128], fp32, tag="ident")
            onec3 = sb.tile([3, 1], fp32, tag="onec3")
            nc.sync.dma_start(wb1_t[:], WB1[:])
            nc.sync.dma_start(wb2_t[:], WB2[:])
            nc.sync.dma_start(wb3_t[:], WB3[:])
            nc.sync.dma_start(wnf1_t[:], Wnf1[:])
            nc.sync.dma_start(wnf2_t[:], Wnf2[:])
            nc.sync.dma_start(wnf3a_t[:], Wnf3[0:128, :])
            nc.sync.dma_start(wnf3b_t[:], Wnf3[128:256, :])
            nc.sync.dma_start(cw1_t[:], CW1[:])
            nc.sync.dma_start(cw2_t[:], CW2[:])
            nc.sync.dma_start(cw3_t[:], CW3[:])
            nc.sync.dma_start(wfi2_t[:], Wfi2[:])
            nc.sync.dma_start(wfi3_t[:], Wfi3[:])
            nc.sync.dma_start(wma_t[:], Wm[0:128, :])
            nc.sync.dma_start(wmb_t[:], Wm[128:256, :])
            nc.sync.dma_start(wl_t[:], Wl[:])
            nc.sync.dma_start(bmT_t[:], bmT[:])
            nc.sync.dma_start(blT_t[:], blT[:])
            make_identity(nc, ident[:])
            nc.vector.memset(onec3[:], -1.0)  # for -|q|^2 row sums
            nc.gpsimd.load_library(library_config.mlp)
            last_rd = {}
            last_rW = {}

            # per-batch state tiles; t loop outer so the two batch chains interleave
            q5_b, q4_b, key5_b, fALL_b, aux_b = [], [], [], [], []
            for b in range(BPC):
                q5_b.append([sb.tile([5, N], fp32, tag=f"q5_{b}_{i}", name=f"q5_{b}_{i}") for i in range(2)])
                q4_b.append([sb.tile([4, N], bf16, tag=f"q4_{b}_{i}", name=f"q4_{b}_{i}") for i in range(2)])
                key5_b.append(sb.tile([5, N], fp32, tag=f"key5_{b}", name=f"key5_{b}"))
                fALL_b.append(sb.tile([128, 4, N], bf16, tag=f"fALL_{b}", name=f"fALL_{b}"))
                nc.vector.memset(fALL_b[b][:], 0.0)

            def qidx(t):
                return t % 2 if t < HALF else (t + 1) % 2

            for t in range(SEQ):
                for b in range(BPC):
                    q5, q4 = q5_b[b], q4_b[b]
                    key5, fALL = key5_b[b], fALL_b[b]
                    tab = tabs[b][t % 2]
                    qi = qidx(t)
                    qt5, qt4 = q5[qi], q4[qi]
                    if t < HALF:
                        base = b * SEQ + t
                        nc.sync.dma_start(qt5[:], q5_l[base, :, :])
                        nc.sync.dma_start(qt4[:], q4_l[base, :, :])
                        kbase = b * SEQ + max(t - 1, 0)
                        nc.sync.dma_start(key5[:], k5_l[kbase, :, :])
                        kt4 = q4[max(t - 1, 0) % 2]
                    elif t == HALF:
                        # q stays = frame5 tiles; keys = frame5 too
                        nc.sync.dma_start(key5[:], k5_l[b * SEQ + HALF - 1, :, :])
                        kt4 = q4[(HALF - 1) % 2]
                    else:
                        # key tiles derived at the end of step t-1
                        kt4 = q4[qidx(t - 1)]

                    # ---- A table: A[key] = [k;1]@WB + f@Wnf, per key chunk ----
                    wr_insts = []
                    for j in range(NCHUNK):
                        jj = slice(j * 128, (j + 1) * 128)
                        a_ps = ps.tile([128, CTOT], fp32, tag="a_ps", bufs=2)
                        nc.tensor.matmul(
                            a_ps[:, 0:64], kt4[:, jj], wb1_t[:], start=True, stop=False
                        )
                        nc.tensor.matmul(
                            a_ps[:, 0:64], fALL[0:64, 0, jj], wnf1_t[:],
                            start=False, stop=True,
                        )
                        nc.tensor.matmul(
                            a_ps[:, 64:192], kt4[:, jj], wb2_t[:], start=True, stop=False
                        )
                        nc.tensor.matmul(
                            a_ps[:, 64:192], fALL[:, 1, jj], wnf2_t[:],
                            start=False, stop=True,
                        )
                        nc.tensor.matmul(
                            a_ps[:, 192:448], kt4[:, jj], wb3_t[:], start=True, stop=False
                        )
                        nc.tensor.matmul(
                            a_ps[:, 192:448], fALL[:, 2, jj], wnf3a_t[:],
                            start=False, stop=False,
                        )
                        nc.tensor.matmul(
                            a_ps[:, 192:448], fALL[:, 3, jj], wnf3b_t[:],
                            start=False, stop=True,
                        )
                        a_sb = sb.tile([128, CTOT], bf16, tag="a_sb", bufs=4)
                        nc.scalar.copy(a_sb[:], a_ps[:])
                        w = nc.sync.dma_start(tab[jj, 0:448], a_sb[:])
                        wr_insts.append(w.ins)

                    # ---- d2 + top-8 per chunk; ordinal o = c*8+k (chunk-outer);
                    # per 2-chunk group: rewrap indices + dma_gather ----
                    Dt = Dts[b][t % 2]
                    Wt = Wts[b][t % 2]
                    idxu = sb.tile([128, NCHUNK, K], u16, tag=f"idxu_{b}", bufs=2)
                    addA = sb.tile([128, NCHUNK, K], bf16, tag=f"addA_{b}", bufs=2)
                    dst = sb.tile([128, 32, 512], bf16, tag=f"dst_{b}", bufs=1)
                    gather_insts = []
                    for j in range(NCHUNK):
                        jj = slice(j * 128, (j + 1) * 128)
                        d2_ps = ps.tile([128, N], fp32, tag="d2_ps", bufs=3)
                        nc.tensor.matmul(
                            d2_ps[:], qt5[:, jj], key5[:], start=True, stop=True
                        )
                        d2h = sb.tile([128, N], fp16, tag="d2h", bufs=4)
                        nc.scalar.copy(d2h[:], d2_ps[:])
                        vals = sb.tile([128, K], fp16, tag="vals", bufs=4)
                        idx8 = sb.tile([128, K], u16, tag="idx8", bufs=4)
                        nc.vector.max(vals[:], d2h[:])
                        nc.vector.max_index(idx8[:], vals[:], d2h[:])
                        nc.vector.tensor_copy(idxu[:, j, :], idx8[:])
                        # addend = (vals < -r^2) * -BIG  (0 for valid)
                        nc.vector.tensor_scalar(
                            addA[:, j, :], vals[:], -R1SQ, -BIG,
                            op0=mybir.AluOpType.is_lt, op1=mybir.AluOpType.mult,
                        )
                    # full-step index rewrap (validated path), then 2 group gathers
                    wD = nc.sync.dma_start(Dt[:, :], idxu[:])
                    for r in last_rd.get((b, t % 2), []):
                        add_dep_helper(wD.ins, r, reason="D reuse after prior read")
                    w16 = sb.tile([16, 32, 8], u16, tag=f"w16_{b}", bufs=2)
                    rd = nc.sync.dma_start(
                        w16[:], Dt[:, :].rearrange("(ph pp) o -> pp o ph", pp=16)
                    )
                    add_dep_helper(rd.ins, wD.ins, reason="read D after write")
                    last_rd[(b, t % 2)] = [rd.ins]
                    wW = nc.sync.dma_start(Wt[:, :], w16[:])
                    for r in last_rW.get((b, t % 2), []):
                        add_dep_helper(wW.ins, r, reason="W reuse after prior read")
                    w128 = sb.tile([128, 256], u16, tag=f"w128_{b}", bufs=2)
                    rW = nc.sync.dma_start(
                        w128[:],
                        Wt[:, :].unsqueeze(0).broadcast_to((8, 16, 256)),
                    )
                    add_dep_helper(rW.ins, wW.ins, reason="read W after write")
                    last_rW[(b, t % 2)] = [rW.ins]
                    for cc in range(2 * NCHUNK):
                        co = slice(cc * 4, (cc + 1) * 4)
                        gi = nc.gpsimd.dma_gather(
                            out_ap=dst[:, co, :],
                            in_ap=tab[:],
                            idxs_ap=w128[:, cc * 32 : (cc + 1) * 32].bitcast(
                                mybir.dt.int16
                            ),
                            num_idxs=4 * 128,
                            num_idxs_reg=4 * 128,
                            elem_size=512,
                            single_packet=False,
                        )
                        for w in wr_insts:
                            add_dep_helper(gi.ins, w, reason="gather after tab write")
                        gather_insts.append(gi)

                    # per-chunk max-pools (rows o = c*8 + k)
                    mAll1 = sb.tile([128, 4, 64], fp32, tag=f"mAll1_{b}", bufs=2)
                    mAll23 = sb.tile([128, 4, 384], fp32, tag=f"mAll23_{b}", bufs=2)
                    for j in range(NCHUNK):
                        j8 = j * 8
                        g1m = sb.tile([128, 8, 64], bf16, tag="g1m", bufs=2)
                        nc.vector.tensor_tensor(
                            g1m[:], dst[:, j8 : j8 + 8, 0:64],
                            addA[:, j, :].unsqueeze(2).broadcast_to((128, 8, 64)),
                            op=mybir.AluOpType.add,
                        )
                        p1a = sb.tile([128, 4, 64], bf16, tag="p1a", bufs=2)
                        nc.vector.tensor_tensor(
                            p1a[:], g1m[:, 0:4, :], g1m[:, 4:8, :],
                            op=mybir.AluOpType.max,
                        )
                        p1b = sb.tile([128, 2, 64], bf16, tag="p1b", bufs=2)
                        nc.vector.tensor_tensor(
                            p1b[:], p1a[:, 0:2, :], p1a[:, 2:4, :],
                            op=mybir.AluOpType.max,
                        )
                        p1c = sb.tile([128, 64], bf16, tag="p1c", bufs=2)
                        nc.vector.tensor_tensor(
                            p1c[:], p1b[:, 0, :], p1b[:, 1, :],
                            op=mybir.AluOpType.max,
                        )
                        nc.vector.tensor_tensor(
                            mAll1[:, j, :], p1c[:], dst[:, j8, 0:64],
                            op=mybir.AluOpType.max,
                        )
                        p2a = sb.tile([128, 4, 384], bf16, tag="p2a", bufs=2)
                        nc.vector.tensor_tensor(
                            p2a[:], dst[:, j8 : j8 + 4, 64:448],
                            dst[:, j8 + 4 : j8 + 8, 64:448],
                            op=mybir.AluOpType.max,
                        )
                        p2b = sb.tile([128, 2, 384], bf16, tag="p2b", bufs=2)
                        nc.vector.tensor_tensor(
                            p2b[:], p2a[:, 0:2, :], p2a[:, 2:4, :],
                            op=mybir.AluOpType.max,
                        )
                        nc.vector.tensor_tensor(
                            mAll23[:, j, :], p2b[:, 0, :], p2b[:, 1, :],
                            op=mybir.AluOpType.max,
                        )

                    # ---- transposed C + m^T accumulate, per cell, per chunk ----
                    for j in range(NCHUNK):
                        jj = slice(j * 128, (j + 1) * 128)
                        ct = ps.tile([128, 4, 128], fp32, tag="ct", bufs=2)
                        # cell1
                        nc.tensor.matmul(
                            ct[0:64, 0, :], cw1_t[:], qt4[0:3, jj], start=True, stop=False
                        )
                        nc.tensor.matmul(
                            ct[0:64, 0, :], mAll1[:, j, :], ident[:],
                            is_transpose=True, start=False, stop=True,
                        )
                        nc.scalar.copy(fALL[0:64, 0, jj], ct[0:64, 0, :])
                        # cell2
                        nc.tensor.matmul(
                            ct[:, 1, :], cw2_t[:], qt4[0:3, jj], start=True, stop=False
                        )
                        nc.tensor.matmul(
                            ct[:, 1, :], wfi2_t[:], fALL[0:64, 0, jj],
                            start=False, stop=False,
                        )
                        nc.tensor.matmul(
                            ct[:, 1, :], mAll23[:, j, 0:128], ident[:],
                            is_transpose=True, start=False, stop=True,
                        )
                        nc.scalar.copy(fALL[:, 1, jj], ct[:, 1, :])
                        # cell3
                        nc.tensor.matmul(
                            ct[:, 2, :], cw3_t[:, 0:128], qt4[0:3, jj],
                            start=True, stop=False,
                        )
                        nc.tensor.matmul(
                            ct[:, 2, :], wfi3_t[:, 0:128], fALL[:, 1, jj],
                            start=False, stop=False,
                        )
                        nc.tensor.matmul(
                            ct[:, 2, :], mAll23[:, j, 128:256], ident[:],
                            is_transpose=True, start=False, stop=True,
                        )
                        nc.tensor.matmul(
                            ct[:, 3, :], cw3_t[:, 128:256], qt4[0:3, jj],
                            start=True, stop=False,
                        )
                        nc.tensor.matmul(
                            ct[:, 3, :], wfi3_t[:, 128:256], fALL[:, 1, jj],
                            start=False, stop=False,
                        )
                        nc.tensor.matmul(
                            ct[:, 3, :], mAll23[:, j, 256:384], ident[:],
                            is_transpose=True, start=False, stop=True,
                        )
                        nc.scalar.copy(fALL[:, 2:4, jj], ct[:, 2:4, :])

                    if t >= HALF:
                        # motion MLP from f3 (fALL slices 2:4)
                        aux = ps.tile([128, N], fp32, tag="aux", bufs=1)
                        nc.tensor.matmul(
                            aux[0:H, :], wma_t[:], fALL[:, 2, :], start=True, stop=False
                        )
                        nc.tensor.matmul(
                            aux[0:H, :], wmb_t[:], fALL[:, 3, :], start=False, stop=True
                        )
                        hm_sb = sb.tile([H, N], bf16, tag=f"hm_sb_{b}")
                        nc.scalar.activation(
                            hm_sb[:], aux[0:H, :],
                            mybir.ActivationFunctionType.Relu,
                            bias=bmT_t[:], scale=1.0,
                        )
                        nc.tensor.matmul(
                            aux[0:3, :], wl_t[:], hm_sb[:], start=True, stop=True
                        )
                        # next query = q + motion + bl
                        nq5 = q5[qidx(t + 1) if t + 1 < SEQ else (t % 2)]
                        nq4 = q4[qidx(t + 1) if t + 1 < SEQ else (t % 2)]
                        nc.vector.scalar_tensor_tensor(
                            nq5[0:3, :], aux[0:3, :], blT_t[:], qt5[0:3, :],
                            op0=mybir.AluOpType.add, op1=mybir.AluOpType.add,
                        )
                        row = (b * HALF + (t - HALF)) * 3
                        nc.sync.dma_start(preds[row : row + 3, :], nq5[0:3, :])
                        if t + 1 < SEQ:
                            # derive next-step q aux rows and key tiles
                            sq3 = sb.tile([3, N], fp32, tag=f"sq3_{b}", bufs=2)
                            nc.vector.tensor_tensor(
                                sq3[:], nq5[0:3, :], nq5[0:3, :],
                                op=mybir.AluOpType.mult,
                            )
                            nc.tensor.matmul(
                                aux[0:1, :], onec3[:], sq3[:], start=True, stop=True
                            )
                            nqs_sb = sb.tile([1, N], fp32, tag="nqs_sb", bufs=2)
                            nc.scalar.copy(nqs_sb[:], aux[0:1, :])
                            nc.sync.dma_start(nq5[4:5, :], nqs_sb[:])
                            nc.scalar.copy(nq4[0:3, :], nq5[0:3, :])
                            # key tiles for t+1 = current q (scaled forms)
                            nc.vector.tensor_scalar(
                                key5[0:3, :], qt5[0:3, :], 2.0, None,
                                op0=mybir.AluOpType.mult,
                            )
                            nc.sync.dma_start(key5[3:4, :], qt5[4:5, :])

    nc.finalize()
    _CACHE["nc"] = nc
    return nc


def _prep_weights(inputs):
    W1, b1 = inputs["W1"], inputs["b1"]
    W2, b2 = inputs["W2"], inputs["b2"]
    W3, b3 = inputs["W3"], inputs["b3"]

    def wb(W, bvec, cout):
        return np.ascontiguousarray(
            np.concatenate([W[0:3], bvec[None, :]], axis=0), np.float32
        )


    return {
        "WB1": wb(W1, b1, 64),
        "WB2": wb(W2, b2, 128),
        "WB3": wb(W3, b3, 256),
        "Wnf1": np.ascontiguousarray(W1[3:67], np.float32),
        "Wnf2": np.ascontiguousarray(W2[67:195], np.float32),
        "Wnf3": np.ascontiguousarray(W3[131:387], np.float32),
        "CW1": np.ascontiguousarray(-W1[0:3], np.float32),
        "CW2": np.ascontiguousarray(-W2[0:3], np.float32),
        "CW3": np.ascontiguousarray(-W3[0:3], np.float32),
        "Wfi2": np.ascontiguousarray(W2[3:67], np.float32),
        "Wfi3": np.ascontiguousarray(W3[3:131], np.float32),
        "Wm": np.ascontiguousarray(inputs["Wm"], np.float32),
        "Wl": np.ascontiguousarray(inputs["Wl"], np.float32),
        "bmT": np.ascontiguousarray(inputs["bm"][:, None], np.float32),
        "blT": np.ascontiguousarray(inputs["bl"][:, None], np.float32),
    }


def _to_bf16(x):
    import ml_dtypes

    return np.asarray(x, np.float32).astype(ml_dtypes.bfloat16)


def _prep_frames(frames):
    # frames (BPC, SEQ, N, 3) for one core -> q5/k5 fp32 and q4 bf16 rows
    x = frames.transpose(0, 1, 3, 2)  # (BPC, SEQ, 3, N)
    ssq = np.sum(x * x, axis=2, keepdims=True)  # (BPC, SEQ, 1, N)
    ones = np.ones_like(ssq)
    q5 = np.concatenate([x, ones, -ssq], axis=2).reshape(BPC * SEQ, 5, N)
    k5 = np.concatenate([2.0 * x, -ssq, ones], axis=2).reshape(BPC * SEQ, 5, N)
    q4 = np.concatenate([x, ones], axis=2).reshape(BPC * SEQ, 4, N)
    return (
        np.ascontiguousarray(q5, np.float32),
        np.ascontiguousarray(k5, np.float32),
        _to_bf16(np.ascontiguousarray(q4, np.float32)),
    )


def _run(inputs, **spmd_kwargs):
    nc = _build()
    frames = np.asarray(inputs["frames"], np.float32)  # (16,12,512,3)
    wd = _prep_weights(
        {k: np.asarray(v, np.float32) for k, v in inputs.items() if k != "frames"}
    )
    bf_names = {
        "WB1", "WB2", "WB3", "Wnf1", "Wnf2", "Wnf3", "CW1", "CW2", "CW3",
        "Wfi2", "Wfi3", "Wm", "Wl",
    }
    wd = {k: (_to_bf16(v) if k in bf_names else v) for k, v in wd.items()}
    in_maps = []
    for c in range(NCORES):
        d = dict(wd)
        q5c, k5c, q4c = _prep_frames(frames[c * BPC : (c + 1) * BPC])
        d["q5_l"] = q5c
        d["k5_l"] = k5c
        d["q4_l"] = q4c
        in_maps.append(d)
    res = run_bass_kernel_spmd(nc, in_maps, list(range(NCORES)), **spmd_kwargs)
    outs = []
    for c in range(NCORES):
        p = np.asarray(res.results[c]["preds"]).reshape(BPC, HALF, 3, N)
        outs.append(p.transpose(0, 1, 3, 2))  # (2,6,512,3)
    return np.ascontiguousarray(np.concatenate(outs, axis=0), np.float32), res


def kernel(**inputs):
    return _run(inputs)[0]
